# revision 1
# baseline (speedup 1.0000x reference)
"""Trainium2 Bass kernel for nn_MeinBlock (2-layer GCN w/ PReLU + BatchNorm).

Computation (reference):
    h = PReLU(x, a1); h = BN(h, gamma, beta)
    h = GCNConv(h, W1, b1, edges); h = PReLU(h, a2)
    out = GCNConv(h, W2, b2, edges)

GCNConv(h) = dinv * segsum_dst(g[src]) + g*dinv + b   where
    g = dinv * (h @ W),  dinv = deg^-1/2, deg = 1 + indegree.
(The self-loop term (h@W)/deg equals g*dinv.)

Distribution: nodes sharded 8 ways (dst-partitioned edges per the hint).
Each core builds its g-shard, an AllGather forms the full bf16 gather
table, dma_gather fetches messages (grouped by src shard so indices fit
int16), dma_scatter_add (CCE) accumulates into per-core DRAM accumulators.
Duplicate destinations within one scatter call would race in the SDMA CCE,
so edges are split into occurrence rounds (round r = r-th edge of its dst
within its group); rounds alternate between two accumulators.
BN batch stats are combined with a tiny AllReduce.
"""

import os
import sys
from contextlib import ExitStack

import numpy as np

sys.path.insert(0, "/opt/trn_rl_repo")

from concourse import bacc, bass, mybir, tile  # noqa: E402
from concourse import bass_utils as _bu  # noqa: E402
from concourse.bass_utils import run_bass_kernel_spmd  # noqa: E402
from concourse.masks import make_identity  # noqa: E402

# The image's antenv lacks axon_hooks; register the NTFF profile hook
# ourselves so trace=True can report HW exec time. Harmless if it fails.
def _install_ntff_hook():
    import types

    try:
        import antenv.axon_hooks  # noqa: F401
        return
    except ImportError:
        pass
    try:
        import antenv
        from trn_agent_boot.trn_boot import _ntff_profile_via_ctypes

        hook = _ntff_profile_via_ctypes("/opt/axon/libaxon_pjrt.so")
        mod = types.ModuleType("antenv.axon_hooks")
        mod.get_axon_ntff_profile_hook = lambda: hook
        mod.set_axon_ntff_profile_hook = lambda h: None
        sys.modules["antenv.axon_hooks"] = mod
        antenv.axon_hooks = mod
    except Exception:
        pass


_install_ntff_hook()
_bu.upload_artifacts = lambda tmpdir: tmpdir  # no artifact bucket here

F32 = mybir.dt.float32
BF16 = mybir.dt.bfloat16
I16 = mybir.dt.int16

P = 128          # partitions
D = 128          # feature dim
NC = 8           # cores
EPS = 1e-5
DUMMYROWS = 128  # scatter pad target rows appended to each accumulator


def _ceil(a, b):
    return -(-a // b)


def _rup(a, b):
    return _ceil(a, b) * b


# --------------------------------------------------------------------------
# Host-side edge plan (pure index manipulation = the sharding step)
# --------------------------------------------------------------------------
class EdgePlan:
    """Per-core gather/scatter index tensors + static layout metadata."""

    def __init__(self, src, dst, n_nodes):
        shard = n_nodes // NC
        self.shard = shard
        deg = np.bincount(dst, minlength=n_nodes).astype(np.float64) + 1.0
        self.dinv = (1.0 / np.sqrt(deg)).astype(np.float32)

        per_core = []  # (j_arr, r_arr, src_local, dst_local) sorted by (j, r)
        maxlen = np.zeros((NC, 64), dtype=np.int64)  # [j, r] -> max count
        maxr = np.zeros(NC, dtype=np.int64)
        for c in range(NC):
            m = (dst // shard) == c
            es, ed = src[m], dst[m]
            dl = (ed - c * shard).astype(np.int64)
            j = (es // shard).astype(np.int64)
            sl = (es - j * shard).astype(np.int64)
            # occurrence round of each edge's dst within its group j
            order = np.lexsort((dl, j))
            j_s, dl_s, sl_s = j[order], dl[order], sl[order]
            # cumcount within equal (j, dst) runs
            key = j_s * shard + dl_s
            first = np.ones(len(key), dtype=bool)
            first[1:] = key[1:] != key[:-1]
            run_id = np.cumsum(first) - 1
            run_start = np.flatnonzero(first)
            r = np.arange(len(key)) - run_start[run_id]
            # final order: by (j, r)
            order2 = np.lexsort((r, j_s))
            j_f, r_f = j_s[order2], r[order2]
            per_core.append((j_f, r_f, sl_s[order2], dl_s[order2]))
            for jj in range(NC):
                mj = j_f == jj
                if mj.any():
                    rj = r_f[mj]
                    maxr[jj] = max(maxr[jj], rj.max() + 1)
                    cnt = np.bincount(rj)
                    maxlen[jj, : len(cnt)] = np.maximum(maxlen[jj, : len(cnt)], cnt)

        # static padded layout shared by all cores
        self.rounds = []  # per group j: list of (offset, padded_len)
        self.caps = []    # per group j: total padded length
        off = 0
        for jj in range(NC):
            rl = []
            goff = off
            for rr in range(int(maxr[jj])):
                ln = int(_rup(max(int(maxlen[jj, rr]), 1), P))
                rl.append((off - goff, ln))
                off += ln
            self.rounds.append(rl)
            self.caps.append(off - goff)
        self.sumcap = off

        # fill per-core padded index arrays
        self.src16 = np.zeros((NC, P, self.sumcap // 16), dtype=np.int16)
        self.dst16 = np.zeros((NC, P, self.sumcap // 16), dtype=np.int16)
        for c in range(NC):
            j_f, r_f, sl_f, dl_f = per_core[c]
            sarr = np.zeros(self.sumcap, dtype=np.int16)
            darr = np.zeros(self.sumcap, dtype=np.int16)
            goff = 0
            for jj in range(NC):
                for rr, (roff, rlen) in enumerate(self.rounds[jj]):
                    mjr = (j_f == jj) & (r_f == rr)
                    n = int(mjr.sum())
                    assert n <= rlen
                    base = goff + roff
                    sarr[base : base + n] = sl_f[mjr]
                    darr[base : base + n] = dl_f[mjr]
                    npad = rlen - n
                    if npad:
                        sarr[base + n : base + rlen] = 0
                        darr[base + n : base + rlen] = shard + (
                            np.arange(npad) % DUMMYROWS
                        )
                goff += self.caps[jj]
            # wrap: index i -> [i % 16, i // 16], replicated to 128 partitions
            w = sarr.reshape(-1, 16).T
            self.src16[c] = np.tile(w, (8, 1))
            w = darr.reshape(-1, 16).T
            self.dst16[c] = np.tile(w, (8, 1))


# --------------------------------------------------------------------------
# Device program
# --------------------------------------------------------------------------
def build_program(n_nodes, caps, rounds):
    """One SPMD program for all 8 cores. caps/rounds = static edge layout."""
    shard = n_nodes // NC
    shard_pad = _rup(shard, P)
    nt = shard_pad // P                 # 128-node tiles per shard
    sumcap = sum(caps)
    accrows = shard + DUMMYROWS

    nc = bacc.Bacc(
        "TRN2",
        target_bir_lowering=False,
        debug=False,
        num_devices=NC,
        num_swdge_queues=4,
    )

    x_sh = nc.declare_dram_parameter("x_sh", [shard, D], F32, isOutput=False)
    w1 = nc.declare_dram_parameter("w1", [D, D], F32, isOutput=False)
    w2 = nc.declare_dram_parameter("w2", [D, D], F32, isOutput=False)
    b1r = nc.declare_dram_parameter("b1r", [1, D], F32, isOutput=False)
    b2r = nc.declare_dram_parameter("b2r", [1, D], F32, isOutput=False)
    gam = nc.declare_dram_parameter("gam", [D, 1], F32, isOutput=False)
    bet = nc.declare_dram_parameter("bet", [D, 1], F32, isOutput=False)
    a1 = nc.declare_dram_parameter("a1", [1, 1], F32, isOutput=False)
    a2 = nc.declare_dram_parameter("a2", [1, 1], F32, isOutput=False)
    dinv_r = nc.declare_dram_parameter("dinv_r", [1, shard_pad], F32, isOutput=False)
    dinv_c = nc.declare_dram_parameter("dinv_c", [P, nt], F32, isOutput=False)
    src_idx = nc.declare_dram_parameter("src_idx", [P, sumcap // 16], I16, isOutput=False)
    dst_idx = nc.declare_dram_parameter("dst_idx", [P, sumcap // 16], I16, isOutput=False)
    out = nc.declare_dram_parameter("out", [shard, D], F32, isOutput=True)

    g_sh = [nc.dram_tensor(f"g{i}_sh", [shard_pad, D], BF16) for i in (1, 2)]
    g_full = [
        nc.dram_tensor(f"g{i}_full", [n_nodes, D], BF16, addr_space="Shared")
        for i in (1, 2)
    ]
    accs = [
        [nc.dram_tensor(f"acc{i}_{m}", [accrows, D], BF16) for m in range(2)]
        for i in (0, 1)
    ]
    bn_in = nc.dram_tensor("bn_in", [P, 2], F32)
    bn_out = nc.dram_tensor("bn_out", [P, 2], F32, addr_space="Shared")
    dbg_out1 = nc.dram_tensor("dbg_out1", [shard_pad, D], F32)

    CH = 8           # 128-node tiles per big chunk
    CHN = CH * P     # nodes per big chunk (1024)

    with tile.TileContext(nc) as tc, ExitStack() as ctx:
        singles = ctx.enter_context(tc.tile_pool(name="singles", bufs=1))
        big = ctx.enter_context(tc.tile_pool(name="big", bufs=1))
        xin = ctx.enter_context(tc.tile_pool(name="xin", bufs=2))
        work = ctx.enter_context(tc.tile_pool(name="work", bufs=2))
        stream = ctx.enter_context(tc.tile_pool(name="stream", bufs=3))
        gout = ctx.enter_context(tc.tile_pool(name="gout", bufs=3))
        msgs_tp = ctx.enter_context(tc.tile_pool(name="msgs", bufs=8))
        mm_tp = ctx.enter_context(tc.tile_pool(name="mm", bufs=2, space="PSUM"))
        tp_tp = ctx.enter_context(tc.tile_pool(name="tp", bufs=4, space="PSUM"))
        stat_tp = ctx.enter_context(tc.tile_pool(name="stat", bufs=1))

        # ---- constants -------------------------------------------------
        idbf = singles.tile([P, P], BF16)
        make_identity(nc, idbf[:])
        a1c = singles.tile([P, 1], F32)
        nc.sync.dma_start(out=a1c[:], in_=a1[:].to_broadcast([P, 1]))
        a2c = singles.tile([P, 1], F32)
        nc.sync.dma_start(out=a2c[:], in_=a2[:].to_broadcast([P, 1]))
        b1row = singles.tile([P, D], F32)
        nc.sync.dma_start(out=b1row[:], in_=b1r[:].to_broadcast([P, D]))
        b2row = singles.tile([P, D], F32)
        nc.sync.dma_start(out=b2row[:], in_=b2r[:].to_broadcast([P, D]))
        gamc = singles.tile([P, 1], F32)
        nc.sync.dma_start(out=gamc[:], in_=gam[:])
        betc = singles.tile([P, 1], F32)
        nc.sync.dma_start(out=betc[:], in_=bet[:])
        dinvc = singles.tile([P, nt], F32)
        nc.sync.dma_start(out=dinvc[:], in_=dinv_c[:])
        w1f = singles.tile([P, D], F32)
        nc.sync.dma_start(out=w1f[:], in_=w1[:])
        w1b = singles.tile([P, D], BF16)
        nc.vector.tensor_copy(w1b[:], w1f[:])
        w2f = singles.tile([P, D], F32)
        nc.sync.dma_start(out=w2f[:], in_=w2[:])
        w2b = singles.tile([P, D], BF16)
        nc.vector.tensor_copy(w2b[:], w2f[:])
        sidx = singles.tile([P, sumcap // 16], I16)
        nc.sync.dma_start(out=sidx[:], in_=src_idx[:])
        didx = singles.tile([P, sumcap // 16], I16)
        nc.sync.dma_start(out=didx[:], in_=dst_idx[:])
        zt = singles.tile([P, CH // 2, P], BF16)
        nc.vector.memset(zt[:], 0.0)

        hT = big.tile([P, shard_pad], BF16, tag="hbig")

        def zero_acc(acc):
            zn = CH // 2 * P
            for s in range(_ceil(accrows, zn)):
                r0 = s * zn
                rows = min(zn, accrows - r0)
                full, rem = divmod(rows, P)
                if full:
                    dst = acc[r0 : r0 + full * P, :].rearrange(
                        "(t p) f -> p t f", p=P
                    )
                    nc.sync.dma_start(out=dst, in_=zt[:, :full, :])
                if rem:
                    dst2 = acc[r0 + full * P : r0 + rows, :]
                    nc.sync.dma_start(out=dst2, in_=zt[:rem, 0, :])

        def load_node_chunk(dram, r0, rows, dtype, pool):
            """DRAM rows [r0, r0+rows) -> SBUF [128, ceil(rows/128), 128]."""
            full, rem = divmod(rows, P)
            t = pool.tile([P, CH, P], dtype, tag="ld")
            if full:
                src = dram[r0 : r0 + full * P, :].rearrange("(t p) f -> p t f", p=P)
                nc.sync.dma_start(out=t[:, :full, :], in_=src)
            if rem:
                nc.vector.memset(t[:, full, :], 0.0)
                nc.sync.dma_start(
                    out=t[:rem, full, :], in_=dram[r0 + full * P : r0 + rows, :]
                )
            return t

        def transpose_block(src_bf16, ntile, dst_big, col0):
            """node-major [128, ntile, 128] -> dst_big[:, col0 : col0+128*ntile]."""
            for k in range(ntile):
                pt = tp_tp.tile([P, P], BF16, tag="tp")
                nc.tensor.transpose(out=pt[:], in_=src_bf16[:, k, :], identity=idbf[:])
                nc.any.tensor_copy(
                    out=dst_big[:, col0 + k * P : col0 + (k + 1) * P], in_=pt[:]
                )

        def prelu_chunk(x_f32, ac, ntile, out_dtype, pool):
            """max(x, a*x) on [128, ntile, 128]."""
            ax = pool.tile([P, CH, P], F32, tag="ax")
            nc.vector.tensor_scalar_mul(ax[:, :ntile, :], x_f32[:, :ntile, :], ac[:, :1])
            h = pool.tile([P, CH, P], out_dtype, tag="h")
            nc.vector.tensor_tensor(
                out=h[:, :ntile, :],
                in0=x_f32[:, :ntile, :],
                in1=ax[:, :ntile, :],
                op=mybir.AluOpType.max,
            )
            return h

        # ================= conv1 phase A: x -> hT (bf16, feature-major) ====
        nch = _ceil(shard, CHN)
        for s in range(nch):
            r0 = s * CHN
            rows = min(CHN, shard - r0)
            ntile = _ceil(rows, P)
            xt = load_node_chunk(x_sh, r0, rows, F32, xin)
            h = prelu_chunk(xt, a1c, ntile, BF16, work)
            transpose_block(h, ntile, hT, r0)
        if shard_pad > _rup(shard, P):
            nc.vector.memset(hT[:, _rup(shard, P) :], 0.0)

        # ================= BN stats + allreduce ============================
        q = 500 if shard % 500 == 0 else int(np.gcd(shard, 512))
        while shard % q or q > 512:
            q -= 1
        sg = shard // q
        stats = stat_tp.tile([P, sg, 6], F32)
        hT3 = hT[:, :shard].rearrange("p (s q) -> p s q", q=q)
        for i in range(sg):
            nc.vector.bn_stats(out=stats[:, i, :], in_=hT3[:, i, :])
        mv = stat_tp.tile([P, 2], F32)
        nc.vector.bn_aggr(out=mv[:], in_=stats[:])
        # allreduce (mean/8, (var+mean^2)/8)
        ar = stat_tp.tile([P, 2], F32)
        nc.vector.tensor_tensor(
            out=ar[:, 1:2], in0=mv[:, 0:1], in1=mv[:, 0:1], op=mybir.AluOpType.mult
        )
        nc.vector.tensor_tensor(
            out=ar[:, 1:2], in0=ar[:, 1:2], in1=mv[:, 1:2], op=mybir.AluOpType.add
        )
        nc.vector.tensor_scalar_mul(ar[:, 1:2], ar[:, 1:2], 1.0 / NC)
        nc.vector.tensor_scalar_mul(ar[:, 0:1], mv[:, 0:1], 1.0 / NC)
        nc.sync.dma_start(out=bn_in[:], in_=ar[:])
        nc.gpsimd.collective_compute(
            "AllReduce",
            mybir.AluOpType.add,
            replica_groups=[list(range(NC))],
            ins=[bn_in[:]],
            outs=[bn_out[:]],
        )
        st = stat_tp.tile([P, 2], F32)
        nc.sync.dma_start(out=st[:], in_=bn_out[:])
        var = stat_tp.tile([P, 1], F32)
        nc.vector.tensor_tensor(
            out=var[:], in0=st[:, 0:1], in1=st[:, 0:1], op=mybir.AluOpType.mult
        )
        nc.vector.tensor_tensor(
            out=var[:], in0=st[:, 1:2], in1=var[:], op=mybir.AluOpType.subtract
        )
        epst = stat_tp.tile([P, 1], F32)
        nc.vector.memset(epst[:], EPS)
        rstd = stat_tp.tile([P, 1], F32)
        nc.scalar.activation(
            out=rstd[:],
            in_=var[:],
            func=mybir.ActivationFunctionType.Sqrt,
            bias=epst[:],
        )
        nc.vector.reciprocal(out=rstd[:], in_=rstd[:])
        scol = stat_tp.tile([P, 1], F32)
        nc.vector.tensor_tensor(
            out=scol[:], in0=gamc[:], in1=rstd[:], op=mybir.AluOpType.mult
        )
        tcol = stat_tp.tile([P, 1], F32)
        nc.vector.tensor_tensor(
            out=tcol[:], in0=st[:, 0:1], in1=scol[:], op=mybir.AluOpType.mult
        )
        nc.vector.tensor_tensor(
            out=tcol[:], in0=betc[:], in1=tcol[:], op=mybir.AluOpType.subtract
        )

        # ============== shared: hT -> g (normalize? -> dinv -> matmul -> T)
        MC = 512  # nodes per matmul chunk

        def build_g(conv, src_big, wts, g_dst):
            nmc = _ceil(shard_pad, MC)
            for m in range(nmc):
                c0 = m * MC
                cols = min(MC, shard_pad - c0)
                if conv == 1:
                    nh = stream.tile([P, MC], BF16, tag="nh")
                    nc.scalar.activation(
                        out=nh[:, :cols],
                        in_=src_big[:, c0 : c0 + cols],
                        func=mybir.ActivationFunctionType.Identity,
                        bias=tcol[:],
                        scale=scol[:],
                    )
                    base = nh
                else:
                    base = None
                dvb = stream.tile([P, MC], F32, tag="dvb")
                nc.sync.dma_start(
                    out=dvb[:, :cols],
                    in_=dinv_r[0:1, c0 : c0 + cols].to_broadcast([P, cols]),
                )
                dv = stream.tile([P, MC], BF16, tag="dv")
                nc.vector.tensor_tensor(
                    out=dv[:, :cols],
                    in0=(base[:, :cols] if base is not None else src_big[:, c0 : c0 + cols]),
                    in1=dvb[:, :cols],
                    op=mybir.AluOpType.mult,
                )
                mm = mm_tp.tile([P, MC], F32, tag="mm")
                nc.tensor.matmul(
                    out=mm[:, :cols], lhsT=wts[:], rhs=dv[:, :cols], start=True, stop=True
                )
                gT = stream.tile([P, MC], BF16, tag="gT")
                nc.any.tensor_copy(out=gT[:, :cols], in_=mm[:, :cols])
                stg = gout.tile([P, MC // P, P], BF16, tag="stg")
                for k in range(_ceil(cols, P)):
                    pt = tp_tp.tile([P, P], BF16, tag="tp")
                    nc.tensor.transpose(
                        out=pt[:], in_=gT[:, k * P : (k + 1) * P], identity=idbf[:]
                    )
                    nc.any.tensor_copy(out=stg[:, k, :], in_=pt[:])
                rows0 = c0
                rows = min(MC, shard_pad - rows0)
                dst = g_dst[rows0 : rows0 + rows, :].rearrange("(t p) f -> p t f", p=P)
                nc.sync.dma_start(out=dst, in_=stg[:, : rows // P, :])

        # ============== edge phase: gather + scatter rounds ================
        # SWDGE ring holds 1024 descriptors -> <=1024-index calls. Tile hands
        # the 8 DMASW sem lanes to SWDGE ops round-robin in Pool program
        # order, and each sem is locked to one queue; queue_num = (k%8)//2
        # keeps the sem<->queue binding consistent while spreading calls over
        # all 4 rings (SDMA drains rings round-robin -> parallel drain).
        GCH = 1024

        def swq():
            return 0  # rewritten post-scheduling from the assigned DMASW lane

        def edge_phase(g_full_t, acc_pair):
            goff = 0
            sctr = 0
            for j in range(NC):
                cap = caps[j]
                if cap == 0:
                    continue
                # scatter split points: round boundaries + chunk boundaries
                bounds = sorted(
                    {0, cap}
                    | {roff for roff, _ in rounds[j]}
                    | {c for c in range(GCH, cap, GCH)}
                )
                for c0 in range(0, cap, GCH):
                    clen = min(GCH, cap - c0)
                    msgs = msgs_tp.tile([P, GCH // P, P], BF16, tag="msgs")
                    nc.gpsimd.dma_gather(
                        msgs[:, : clen // P, :],
                        g_full_t[j * shard : (j + 1) * shard, :],
                        sidx[:, (goff + c0) // 16 : (goff + c0 + clen) // 16],
                        clen,
                        clen,
                        D,
                        queue_num=swq(),
                        single_packet=True,
                    )
                    subs = [b for b in bounds if c0 <= b <= c0 + clen]
                    for a, b in zip(subs, subs[1:]):
                        acc = acc_pair[sctr % len(acc_pair)]
                        sctr += 1
                        nc.gpsimd.dma_scatter_add(
                            acc[:],
                            msgs[:, (a - c0) // P : (b - c0) // P, :],
                            didx[:, (goff + a) // 16 : (goff + b) // 16],
                            b - a,
                            b - a,
                            D,
                            queue_num=swq(),
                            single_packet=True,
                        )
                goff += cap

        # ============== readback: out_nm = dinv*(acc0+acc1+g_own) + brow ===
        def readback(acc_pair, g_own, brow, store_out, prelu_a, dst_big):
            for s in range(nch):
                r0 = s * CHN
                rows = min(CHN, shard - r0)
                ntile = _ceil(rows, P)
                at0 = xin.tile([P, CH, P], BF16, tag="at0")
                src = acc_pair[0][r0 : r0 + ntile * P, :].rearrange(
                    "(t p) f -> p t f", p=P
                )
                nc.sync.dma_start(out=at0[:, :ntile, :], in_=src)
                at1 = xin.tile([P, CH, P], BF16, tag="at1")
                src = acc_pair[1][r0 : r0 + ntile * P, :].rearrange(
                    "(t p) f -> p t f", p=P
                )
                nc.sync.dma_start(out=at1[:, :ntile, :], in_=src)
                gt = xin.tile([P, CH, P], BF16, tag="gt")
                src = g_own[r0 : r0 + ntile * P, :].rearrange("(t p) f -> p t f", p=P)
                nc.sync.dma_start(out=gt[:, :ntile, :], in_=src)

                sm = work.tile([P, CH, P], F32, tag="sm")
                nc.vector.tensor_tensor(
                    out=sm[:, :ntile, :],
                    in0=at0[:, :ntile, :],
                    in1=at1[:, :ntile, :],
                    op=mybir.AluOpType.add,
                )
                nc.vector.tensor_tensor(
                    out=sm[:, :ntile, :],
                    in0=sm[:, :ntile, :],
                    in1=gt[:, :ntile, :],
                    op=mybir.AluOpType.add,
                )
                # * dinv (per-node = per (partition, tile)) via stride-0 f bcast
                dv_ap = bass.AP(
                    tensor=dinvc.tensor,
                    offset=dinvc.offset + s * CH,
                    ap=[list(dinvc.ap[0]), [1, ntile], [0, P]],
                )
                nc.vector.tensor_tensor(
                    out=sm[:, :ntile, :],
                    in0=sm[:, :ntile, :],
                    in1=dv_ap,
                    op=mybir.AluOpType.mult,
                )
                # + b row (replicated tile; bcast over the tile dim only)
                br_ap = bass.AP(
                    tensor=brow.tensor,
                    offset=brow.offset,
                    ap=[list(brow.ap[0]), [0, ntile], [1, P]],
                )
                ot = sm
                nc.vector.tensor_tensor(
                    out=ot[:, :ntile, :],
                    in0=sm[:, :ntile, :],
                    in1=br_ap,
                    op=mybir.AluOpType.add,
                )
                if store_out:
                    full, rem = divmod(rows, P)
                    if full:
                        dst = out[r0 : r0 + full * P, :].rearrange(
                            "(t p) f -> p t f", p=P
                        )
                        nc.sync.dma_start(out=dst, in_=ot[:, :full, :])
                    if rem:
                        nc.sync.dma_start(
                            out=out[r0 + full * P : r0 + rows, :],
                            in_=ot[:rem, full, :],
                        )
                else:
                    dbg_dst = dbg_out1[r0 : r0 + ntile * P, :].rearrange(
                        "(t p) f -> p t f", p=P
                    )
                    nc.sync.dma_start(out=dbg_dst, in_=ot[:, :ntile, :])
                    h2 = prelu_chunk(ot, prelu_a, ntile, BF16, work)
                    transpose_block(h2, ntile, dst_big, r0)

        # =================== schedule both convs ===========================
        for m in range(2):
            zero_acc(accs[0][m])
        build_g(1, hT, w1b, g_sh[0])
        nc.gpsimd.collective_compute(
            "AllGather",
            mybir.AluOpType.bypass,
            replica_groups=[list(range(NC))],
            ins=[g_sh[0][:shard, :]],
            outs=[g_full[0][:]],
        )
        edge_phase(g_full[0], accs[0])
        h2T = big.tile([P, shard_pad], BF16, tag="hbig")
        readback(accs[0], g_sh[0], b1row, False, a2c, h2T)
        if shard_pad > _rup(shard, P):
            nc.vector.memset(h2T[:, _rup(shard, P) :], 0.0)

        for m in range(2):
            zero_acc(accs[1][m])
        build_g(2, h2T, w2b, g_sh[1])
        nc.gpsimd.collective_compute(
            "AllGather",
            mybir.AluOpType.bypass,
            replica_groups=[list(range(NC))],
            ins=[g_sh[1][:shard, :]],
            outs=[g_full[1][:]],
        )
        edge_phase(g_full[1], accs[1])
        readback(accs[1], g_sh[1], b2row, True, None, None)

    # Spread SWDGE calls over the 4 rings, consistent with Tile's DMASW sem
    # lane assignment (lane k <-> queue k//2) so each sem stays locked to
    # one queue while the rings drain in parallel.
    from concourse.tile_sem_assignment import PROC_NAME_TO_IDX

    lane_of = {PROC_NAME_TO_IDX[f"DMASW{k}"]: k for k in range(8)}
    for inst in nc.inst_map.values():
        if isinstance(inst, (mybir.InstDMAGatherAnt, mybir.InstDMAScatterAddAnt)):
            proc = getattr(inst, "bass_scheduled_proc", None)
            if proc in lane_of:
                inst.queue_num = lane_of[proc] // 2

    nc.compile()
    return nc


# --------------------------------------------------------------------------
# Host wrapper
# --------------------------------------------------------------------------
def _prep_inputs(x, edge_index, a1, gamma, beta, W1, b1, a2, W2, b2):
    n = x.shape[0]
    shard = n // NC
    shard_pad = _rup(shard, P)
    nt = shard_pad // P
    ei = np.asarray(edge_index).astype(np.int64)
    plan = EdgePlan(ei[0], ei[1], n)

    in_maps = []
    for c in range(NC):
        dv = plan.dinv[c * shard : (c + 1) * shard]
        dinv_r = np.zeros((1, shard_pad), dtype=np.float32)
        dinv_r[0, :shard] = dv
        dvp = np.zeros(shard_pad, dtype=np.float32)
        dvp[:shard] = dv
        dinv_c = np.ascontiguousarray(dvp.reshape(nt, P).T)  # [p,t]=dinv[t*128+p]
        in_maps.append(
            dict(
                x_sh=np.ascontiguousarray(x[c * shard : (c + 1) * shard]).astype(
                    np.float32
                ),
                w1=np.asarray(W1, dtype=np.float32),
                w2=np.asarray(W2, dtype=np.float32),
                b1r=np.asarray(b1, dtype=np.float32).reshape(1, D),
                b2r=np.asarray(b2, dtype=np.float32).reshape(1, D),
                gam=np.asarray(gamma, dtype=np.float32).reshape(D, 1),
                bet=np.asarray(beta, dtype=np.float32).reshape(D, 1),
                a1=np.asarray(a1, dtype=np.float32).reshape(1, 1),
                a2=np.asarray(a2, dtype=np.float32).reshape(1, 1),
                dinv_r=dinv_r,
                dinv_c=dinv_c,
                src_idx=plan.src16[c],
                dst_idx=plan.dst16[c],
            )
        )
    return plan, in_maps


_PROG_CACHE = {}


def kernel(x, edge_index, a1, gamma, beta, W1, b1, a2, W2, b2, _trace=False):
    x = np.asarray(x)
    n = x.shape[0]
    plan, in_maps = _prep_inputs(
        x, edge_index, a1, gamma, beta, W1, b1, a2, W2, b2
    )
    key = (n, tuple(plan.caps), tuple(tuple(r) for r in plan.rounds))
    if key not in _PROG_CACHE:
        _PROG_CACHE[key] = build_program(n, plan.caps, plan.rounds)
    nc = _PROG_CACHE[key]
    res = run_bass_kernel_spmd(
        nc, in_maps, core_ids=list(range(NC)), trace=_trace
    )
    outs = [res.results[c]["out"] for c in range(NC)]
    full = np.concatenate(outs, axis=0).astype(np.float32)
    kernel._last_exec_ns = res.exec_time_ns
    return full



# revision 2
# speedup vs baseline: 1.0102x; 1.0102x over previous
"""Trainium2 Bass kernel for nn_MeinBlock (2-layer GCN w/ PReLU + BatchNorm).

v2 architecture (replaces AllGather + SWDGE scatter_add design):
  - nodes sharded 8 ways; per conv each core builds g = dinv*(h@W) as a
    node-major bf16 table (SBUF + DRAM copy).
  - SENDER-side SWDGE dma_gather reads the out-edge messages (grouped by
    destination core and destination 512-window/256-half-cells, sorted by
    dst) from its own g table in DRAM -> node-major slabs.
  - 4 quarter-wise AllToAll collectives exchange the slabs (pipelined
    against the next quarter's gathers).
  - RECEIVER scatters with one-hot matmuls on the tensor engine: for each
    256-dst half-window a PSUM [128,256] f32 tile accumulates
    msgs^T @ onehot(dst) over all source slabs, plus the self-loop term
    via identity-pattern matmuls from the local table.
  - Flush fuses dinv scaling + bias (+ PReLU for conv1) on DVE.
  - BatchNorm is folded into W1 (W1s = diag(s) W1, r = t^T W1) after a
    tiny AllReduce of batch stats.
No SWDGE scatter, no accumulator zero/readback, no full-table AllGather.
"""

import sys
from contextlib import ExitStack

import numpy as np

sys.path.insert(0, "/opt/trn_rl_repo")

from concourse import bacc, bass, mybir, tile  # noqa: E402
from concourse import bass_utils as _bu  # noqa: E402
from concourse.bass_utils import run_bass_kernel_spmd  # noqa: E402
from concourse.masks import make_identity  # noqa: E402


def _install_ntff_hook():
    import types

    try:
        import antenv.axon_hooks  # noqa: F401
        return
    except ImportError:
        pass
    try:
        import antenv
        from trn_agent_boot.trn_boot import _ntff_profile_via_ctypes

        hook = _ntff_profile_via_ctypes("/opt/axon/libaxon_pjrt.so")
        mod = types.ModuleType("antenv.axon_hooks")
        mod.get_axon_ntff_profile_hook = lambda: hook
        mod.set_axon_ntff_profile_hook = lambda h: None
        sys.modules["antenv.axon_hooks"] = mod
        antenv.axon_hooks = mod
    except Exception:
        pass


_install_ntff_hook()
_bu.upload_artifacts = lambda tmpdir: tmpdir

F32 = mybir.dt.float32
BF16 = mybir.dt.bfloat16
I16 = mybir.dt.int16

P = 128
D = 128
NC = 8
EPS = 1e-5
WIN = 512          # dst window
HALF = 256         # half-window span (matmul moving width)
GCH = 1024         # max idxs per dma_gather call


def _ceil(a, b):
    return -(-a // b)


def _rup(a, b):
    return _ceil(a, b) * b


def _wrap16(idx):
    """[n] -> [128, n//16] SWDGE wrapped int16 (i -> [i%16, i//16], x8)."""
    return np.tile(idx.reshape(-1, 16).T, (8, 1)).astype(np.int16)


# --------------------------------------------------------------------------
# Host-side edge plan
# --------------------------------------------------------------------------
class Plan:
    def __init__(self, src, dst, n):
        S = n // NC
        self.S = S
        SP = _rup(S, P)
        self.SP = SP
        NW = SP // WIN                      # 49 windows per shard
        self.NW = NW
        deg = np.bincount(dst, minlength=n).astype(np.float64) + 1.0
        dinv = (1.0 / np.sqrt(deg)).astype(np.float32)
        self.dinv = dinv

        # quarter split by window index
        wq = [list(range(13)), list(range(13, 26)),
              list(range(26, 39)), list(range(39, NW))]
        self.wq = wq

        j_of = src // S
        c_of = dst // S
        sl = (src - j_of * S).astype(np.int64)
        dl = (dst - c_of * S).astype(np.int64)
        wh_of = dl // HALF                   # half-cell index 0..97

        # counts per (receiver c, sender j, halfcell)
        NH = NW * 2
        cnt = np.zeros((NC, NC, NH), dtype=np.int64)
        np.add.at(cnt, (c_of, j_of, wh_of), 1)
        self.cap = np.array(
            [_rup(max(int(cnt[:, :, k].max()), 0), P) for k in range(NH)],
            dtype=np.int64)
        # per-quarter slab row counts
        self.capq = [int(sum(self.cap[2 * w] + self.cap[2 * w + 1]
                             for w in wq[q])) for q in range(4)]

        # bucket edges by (c, j, halfcell), sorted by dst within
        order = np.lexsort((dl, wh_of, j_of, c_of))
        c_s, j_s, wh_s = c_of[order], j_of[order], wh_of[order]
        sl_s, dl_s = sl[order], dl[order]
        key = ((c_s * NC + j_s) * NH + wh_s)
        starts = np.searchsorted(key, np.arange(NC * NC * NH) * 1, side="left")
        # build per-core send idx streams + per-core dstrel columns
        self.send_idx = []      # per core j: wrapped int16 [128, L/16]
        self.send_calls = []    # per core j: list of (idx_off, nidx, c, q, row_off)
        self.dstrel = []        # per core c: [128, nblocks] f32

        self.nblocks = NC * int(self.cap.sum()) // P
        allkey = key
        for core in range(NC):
            # ---- sender stream for core (as j): ordered (q, c, w in q, h)
            idxs = []
            calls = []
            for q in range(4):
                for c in range(NC):
                    row_off = 0
                    sec = []
                    for w in wq[q]:
                        for h in (0, 1):
                            k = 2 * w + h
                            cap = int(self.cap[k])
                            if cap == 0:
                                continue
                            kk = (c * NC + core) * NH + k
                            a = int(starts[kk])
                            b = int(starts[kk + 1]) if kk + 1 < len(starts) else len(allkey)
                            b = a + int(cnt[c, core, k])
                            seg = sl_s[a:b]
                            pad = np.zeros(cap, dtype=np.int64)
                            pad[: len(seg)] = seg
                            sec.append(pad)
                    sec = np.concatenate(sec) if sec else np.zeros(0, np.int64)
                    # chunk into GCH calls
                    off = 0
                    base = sum(len(x) for x in idxs)
                    while off < len(sec):
                        ln = min(GCH, len(sec) - off)
                        calls.append((base + off, ln, c, q, off))
                        off += ln
                    idxs.append(sec)
            stream = np.concatenate(idxs) if idxs else np.zeros(0, np.int64)
            assert stream.max(initial=0) < 32768
            self.send_idx.append(_wrap16(stream.astype(np.int16)))
            self.send_calls.append(calls)

            # ---- receiver dstrel for core (as c): blocks ordered (q, w, h, j, b)
            cols = []
            for q in range(4):
                for w in wq[q]:
                    for h in (0, 1):
                        k = 2 * w + h
                        cap = int(self.cap[k])
                        for j in range(NC):
                            kk = (core * NC + j) * NH + k
                            a = int(starts[kk])
                            b = a + int(cnt[core, j, k])
                            seg = dl_s[a:b] - (w * WIN + h * HALF)
                            colv = np.full(cap, -1.0, dtype=np.float32)
                            colv[: len(seg)] = seg.astype(np.float32)
                            cols.append(colv.reshape(-1, P).T)  # [128, cap/P]
            self.dstrel.append(np.concatenate(cols, axis=1))
        self.n_send = self.send_idx[0].shape[1] * 16
        for j in range(1, NC):
            assert self.send_idx[j].shape == self.send_idx[0].shape
            assert self.dstrel[j].shape == self.dstrel[0].shape

    def key(self):
        return (self.S, tuple(self.cap.tolist()), self.n_send)


# --------------------------------------------------------------------------
# Device program
# --------------------------------------------------------------------------
def build_program(plan: Plan):
    S, SP, NW = plan.S, plan.SP, plan.NW
    NT = SP // P
    nc = bacc.Bacc(
        "TRN2",
        target_bir_lowering=False,
        debug=False,
        num_devices=NC,
        num_swdge_queues=4,
    )

    x_sh = nc.declare_dram_parameter("x_sh", [S, D], F32, isOutput=False)
    w1 = nc.declare_dram_parameter("w1", [D, D], F32, isOutput=False)
    w2 = nc.declare_dram_parameter("w2", [D, D], F32, isOutput=False)
    b1c = nc.declare_dram_parameter("b1c", [D, 1], F32, isOutput=False)
    b2c = nc.declare_dram_parameter("b2c", [D, 1], F32, isOutput=False)
    gam = nc.declare_dram_parameter("gam", [D, 1], F32, isOutput=False)
    bet = nc.declare_dram_parameter("bet", [D, 1], F32, isOutput=False)
    a1 = nc.declare_dram_parameter("a1", [1, 1], F32, isOutput=False)
    a2 = nc.declare_dram_parameter("a2", [1, 1], F32, isOutput=False)
    dinv_nm = nc.declare_dram_parameter("dinv_nm", [P, NT], F32, isOutput=False)
    dinv_fm = nc.declare_dram_parameter("dinv_fm", [1, SP], F32, isOutput=False)
    sgidx = nc.declare_dram_parameter("sgidx", [P, plan.n_send // 16], I16, isOutput=False)
    dstrel = nc.declare_dram_parameter("dstrel", [P, plan.nblocks], F32, isOutput=False)
    iota_r = nc.declare_dram_parameter("iota_r", [1, HALF], F32, isOutput=False)
    eyes = nc.declare_dram_parameter("eyes", [P, 2 * HALF], F32, isOutput=False)
    out = nc.declare_dram_parameter("out", [S, D], F32, isOutput=True)

    g_dram = [nc.dram_tensor(f"g_dram{i}", [SP, D], BF16) for i in (0, 1)]
    sendb = [[nc.dram_tensor(f"send{i}_{q}", [NC * plan.capq[q], D], BF16)
              for q in range(4)] for i in (0, 1)]
    recvb = [[nc.dram_tensor(f"recv{i}_{q}", [NC * plan.capq[q], D], BF16)
              for q in range(4)] for i in (0, 1)]
    bn_in = nc.dram_tensor("bn_in", [P, 2], F32)
    bn_out = nc.dram_tensor("bn_out", [P, 2], F32, addr_space="Shared")

    CH = 8
    CHN = CH * P

    with tile.TileContext(nc) as tc, ExitStack() as ctx:
        singles = ctx.enter_context(tc.tile_pool(name="singles", bufs=1))
        big = ctx.enter_context(tc.tile_pool(name="big", bufs=1))
        xin = ctx.enter_context(tc.tile_pool(name="xin", bufs=2))
        work = ctx.enter_context(tc.tile_pool(name="work", bufs=2))
        gpool = ctx.enter_context(tc.tile_pool(name="gath", bufs=8))
        slabp = ctx.enter_context(tc.tile_pool(name="slab", bufs=1))
        maskp = ctx.enter_context(tc.tile_pool(name="mask", bufs=4))
        flshp = ctx.enter_context(tc.tile_pool(name="flsh", bufs=2))
        mm_tp = ctx.enter_context(tc.tile_pool(name="mm", bufs=2, space="PSUM"))
        sc_tp = ctx.enter_context(tc.tile_pool(name="sc", bufs=4, space="PSUM"))
        tp_tp = ctx.enter_context(tc.tile_pool(name="tp", bufs=2, space="PSUM"))
        stat_tp = ctx.enter_context(tc.tile_pool(name="stat", bufs=1))

        # ---------------- constants ----------------
        idbf = singles.tile([P, P], BF16)
        make_identity(nc, idbf[:])
        a1c = singles.tile([P, 1], F32)
        nc.sync.dma_start(out=a1c[:], in_=a1[:].to_broadcast([P, 1]))
        a2c = singles.tile([P, 1], F32)
        nc.sync.dma_start(out=a2c[:], in_=a2[:].to_broadcast([P, 1]))
        b1s = singles.tile([P, 1], F32)
        nc.sync.dma_start(out=b1s[:], in_=b1c[:])
        b2s = singles.tile([P, 1], F32)
        nc.sync.dma_start(out=b2s[:], in_=b2c[:])
        gamc = singles.tile([P, 1], F32)
        nc.sync.dma_start(out=gamc[:], in_=gam[:])
        betc = singles.tile([P, 1], F32)
        nc.sync.dma_start(out=betc[:], in_=bet[:])
        dnm = singles.tile([P, NT], F32)
        nc.sync.dma_start(out=dnm[:], in_=dinv_nm[:])
        # dinv per-dst rows are broadcast-loaded from DRAM per flush window
        w1f = singles.tile([P, D], F32)
        nc.sync.dma_start(out=w1f[:], in_=w1[:])
        w2f = singles.tile([P, D], F32)
        nc.sync.dma_start(out=w2f[:], in_=w2[:])
        w2b = singles.tile([P, D], BF16)
        nc.vector.tensor_copy(w2b[:], w2f[:])
        sgix = singles.tile([P, plan.n_send // 16], I16)
        nc.sync.dma_start(out=sgix[:], in_=sgidx[:])
        drel = singles.tile([P, plan.nblocks], F32)
        nc.sync.dma_start(out=drel[:], in_=dstrel[:])
        iot = singles.tile([P, HALF], F32)
        nc.sync.dma_start(out=iot[:], in_=iota_r[:].to_broadcast([P, HALF]))
        eyf = singles.tile([P, 2 * HALF], F32)
        nc.sync.dma_start(out=eyf[:], in_=eyes[:])
        eyb = singles.tile([P, 2 * HALF], BF16)
        nc.vector.tensor_copy(eyb[:], eyf[:])

        h_fm = big.tile([P, SP], BF16, tag="hfm")       # conv input, fm

        def load_node_chunk(dram, r0, rows, dtype, pool):
            full, rem = divmod(rows, P)
            t = pool.tile([P, CH, P], dtype, tag="ld")
            if full:
                src = dram[r0 : r0 + full * P, :].rearrange("(t p) f -> p t f", p=P)
                nc.sync.dma_start(out=t[:, :full, :], in_=src)
            if rem:
                nc.vector.memset(t[:, full, :], 0.0)
                nc.sync.dma_start(
                    out=t[:rem, full, :], in_=dram[r0 + full * P : r0 + rows, :]
                )
            return t

        # ================= phase A: x -> h (prelu) fm + BN stats ===========
        nch = _ceil(S, CHN)
        for s in range(nch):
            r0 = s * CHN
            rows = min(CHN, S - r0)
            ntile = _ceil(rows, P)
            xt = load_node_chunk(x_sh, r0, rows, F32, xin)
            ax = work.tile([P, CH, P], F32, tag="ax")
            nc.vector.tensor_scalar_mul(ax[:, :ntile, :], xt[:, :ntile, :], a1c[:, :1])
            h = work.tile([P, CH, P], BF16, tag="h")
            nc.vector.tensor_tensor(
                out=h[:, :ntile, :], in0=xt[:, :ntile, :], in1=ax[:, :ntile, :],
                op=mybir.AluOpType.max,
            )
            for k in range(ntile):
                pt = tp_tp.tile([P, P], BF16, tag="tp")
                nc.tensor.transpose(out=pt[:], in_=h[:, k, :], identity=idbf[:])
                nc.any.tensor_copy(out=h_fm[:, r0 + k * P : r0 + (k + 1) * P], in_=pt[:])
        if SP > S:
            nc.vector.memset(h_fm[:, S:], 0.0)

        # ================= BN stats + AllReduce ============================
        q = 500 if S % 500 == 0 else int(np.gcd(S, 512))
        while S % q or q > 512:
            q -= 1
        sg = S // q
        stats = stat_tp.tile([P, sg, 6], F32)
        h3 = h_fm[:, :S].rearrange("p (s q) -> p s q", q=q)
        for i in range(sg):
            nc.vector.bn_stats(out=stats[:, i, :], in_=h3[:, i, :])
        mv = stat_tp.tile([P, 2], F32)
        nc.vector.bn_aggr(out=mv[:], in_=stats[:])
        ar = stat_tp.tile([P, 2], F32)
        nc.vector.tensor_tensor(out=ar[:, 1:2], in0=mv[:, 0:1], in1=mv[:, 0:1],
                                op=mybir.AluOpType.mult)
        nc.vector.tensor_tensor(out=ar[:, 1:2], in0=ar[:, 1:2], in1=mv[:, 1:2],
                                op=mybir.AluOpType.add)
        nc.vector.tensor_scalar_mul(ar[:, 1:2], ar[:, 1:2], 1.0 / NC)
        nc.vector.tensor_scalar_mul(ar[:, 0:1], mv[:, 0:1], 1.0 / NC)
        nc.sync.dma_start(out=bn_in[:], in_=ar[:])
        nc.gpsimd.collective_compute(
            "AllReduce", mybir.AluOpType.add,
            replica_groups=[list(range(NC))],
            ins=[bn_in[:]], outs=[bn_out[:]],
        )
        st = stat_tp.tile([P, 2], F32)
        nc.sync.dma_start(out=st[:], in_=bn_out[:])
        var = stat_tp.tile([P, 1], F32)
        nc.vector.tensor_tensor(out=var[:], in0=st[:, 0:1], in1=st[:, 0:1],
                                op=mybir.AluOpType.mult)
        nc.vector.tensor_tensor(out=var[:], in0=st[:, 1:2], in1=var[:],
                                op=mybir.AluOpType.subtract)
        epst = stat_tp.tile([P, 1], F32)
        nc.vector.memset(epst[:], EPS)
        rstd = stat_tp.tile([P, 1], F32)
        nc.scalar.activation(out=rstd[:], in_=var[:],
                             func=mybir.ActivationFunctionType.Sqrt, bias=epst[:])
        nc.vector.reciprocal(out=rstd[:], in_=rstd[:])
        scol = stat_tp.tile([P, 1], F32)
        nc.vector.tensor_tensor(out=scol[:], in0=gamc[:], in1=rstd[:],
                                op=mybir.AluOpType.mult)
        tcol = stat_tp.tile([P, 1], F32)
        nc.vector.tensor_tensor(out=tcol[:], in0=st[:, 0:1], in1=scol[:],
                                op=mybir.AluOpType.mult)
        nc.vector.tensor_tensor(out=tcol[:], in0=betc[:], in1=tcol[:],
                                op=mybir.AluOpType.subtract)
        # W1s = diag(s) W1 (bf16); r = t^T W1  [1,128] f32
        w1sb = singles.tile([P, D], BF16)
        w1sf = stat_tp.tile([P, D], F32)
        nc.vector.tensor_scalar_mul(w1sf[:], w1f[:], scol[:, :1])
        nc.vector.tensor_copy(w1sb[:], w1sf[:])
        rps = mm_tp.tile([P, D], F32, tag="bg")
        nc.tensor.matmul(out=rps[0:1, :], lhsT=tcol[:], rhs=w1f[:], start=True, stop=True)
        rrow1 = singles.tile([1, D], F32)
        nc.any.tensor_copy(out=rrow1[:], in_=rps[0:1, :])
        r_dram = nc.dram_tensor("r_dram", [1, D], F32)
        nc.sync.dma_start(out=r_dram[:], in_=rrow1[:])
        rrep = singles.tile([P, D], F32)
        nc.sync.dma_start(out=rrep[:], in_=r_dram[:].to_broadcast([P, D]))

        # ================= shared pieces ===================================
        def build_g(conv):
            """h_fm -> g_dram (nm bf16, = dinv*(h@W [+r]))."""
            wts = w1sb if conv == 0 else w2b
            gd = g_dram[conv]
            for k0 in range(0, NT, CH):
                kt = min(CH, NT - k0)
                stg = work.tile([P, CH, D], BF16, tag="bgs")
                for ki in range(kt):
                    t = k0 + ki
                    ps = mm_tp.tile([P, D], F32, tag="bg")
                    nc.tensor.matmul(out=ps[:], lhsT=h_fm[:, t * P : (t + 1) * P],
                                     rhs=wts[:], start=True, stop=True)
                    if conv == 0:
                        tmp = work.tile([P, D], F32, tag="bgt")
                        nc.vector.tensor_tensor(out=tmp[:], in0=ps[:], in1=rrep[:],
                                                op=mybir.AluOpType.add)
                        src = tmp
                    else:
                        src = ps
                    nc.vector.tensor_scalar_mul(stg[:, ki, :], src[:],
                                                dnm[:, t : t + 1])
                dst = gd[k0 * P : (k0 + kt) * P, :].rearrange("(t p) f -> p t f", p=P)
                nc.sync.dma_start(out=dst, in_=stg[:, :kt, :])

        def swq():
            return 0

        def sender_gathers(conv, q_filter):
            gd = g_dram[conv]
            for (ioff, nidx, c, q, roff) in plan.send_calls[0]:
                if q != q_filter:
                    continue
                o = gpool.tile([P, GCH // P, P], BF16, tag="o")
                nc.gpsimd.dma_gather(
                    o[:, : nidx // P, :], gd[:],
                    sgix[:, ioff // 16 : (ioff + nidx) // 16],
                    nidx, nidx, D,
                    queue_num=swq(), single_packet=True,
                )
                dst = sendb[conv][q][
                    c * plan.capq[q] + roff : c * plan.capq[q] + roff + nidx, :
                ].rearrange("(t p) f -> p t f", p=P)
                nc.sync.dma_start(out=dst, in_=o[:, : nidx // P, :])

        def scatter_quarter(conv, q, blk_base):
            """Consume recvb[conv][q]: one-hot matmul scatter + flush."""
            capq = plan.capq[q]
            slabs = []
            for j in range(NC):
                sl = slabp.tile([P, capq // P, P], BF16, tag=f"sl{j}")
                src = recvb[conv][q][j * capq : (j + 1) * capq, :].rearrange(
                    "(t p) f -> p t f", p=P)
                nc.scalar.dma_start(out=sl[:], in_=src)
                slabs.append(sl)
            blk = blk_base
            cell_off = 0
            for w in plan.wq[q]:
                for h in (0, 1):
                    k = 2 * w + h
                    cap = int(plan.cap[k])
                    nb = cap // P
                    ps = sc_tp.tile([P, HALF], F32, tag="ps")
                    c0w = w * WIN + h * HALF
                    selfg = maskp.tile([P, 2, P], BF16, tag="selfg")
                    nc.scalar.dma_start(
                        out=selfg[:],
                        in_=g_dram[conv][c0w : c0w + 2 * P, :].rearrange(
                            "(t p) f -> p t f", p=P))
                    nc.tensor.matmul(out=ps[:], lhsT=selfg[:, 0, :],
                                     rhs=eyb[:, :HALF], start=True, stop=False)
                    nc.tensor.matmul(out=ps[:], lhsT=selfg[:, 1, :],
                                     rhs=eyb[:, HALF:], start=False,
                                     stop=(nb == 0))
                    for j in range(NC):
                        for b in range(nb):
                            col = blk + j * nb + b
                            mk = maskp.tile([P, HALF], BF16, tag="mk")
                            d_ap = bass.AP(
                                tensor=drel.tensor,
                                offset=drel.offset + col,
                                ap=[list(drel.ap[0]), [0, HALF]],
                            )
                            nc.vector.tensor_tensor(
                                out=mk[:], in0=d_ap, in1=iot[:],
                                op=mybir.AluOpType.is_equal,
                            )
                            last = (j == NC - 1) and (b == nb - 1)
                            nc.tensor.matmul(
                                out=ps[:],
                                lhsT=slabs[j][:, cell_off // P + b, :],
                                rhs=mk[:], start=False, stop=last,
                            )
                    blk += NC * nb
                    cell_off += cap
                    # -------- flush --------
                    c0 = w * WIN + h * HALF
                    dvb = flshp.tile([P, HALF], F32, tag="dvb")
                    nc.sync.dma_start(
                        out=dvb[:],
                        in_=dinv_fm[0:1, c0 : c0 + HALF].to_broadcast([P, HALF]),
                    )
                    y0 = flshp.tile([P, HALF], F32, tag="y0")
                    nc.vector.tensor_tensor(out=y0[:], in0=ps[:], in1=dvb[:],
                                            op=mybir.AluOpType.mult)
                    if conv == 0:
                        y = flshp.tile([P, HALF], F32, tag="y")
                        nc.scalar.activation(
                            out=y[:], in_=y0[:],
                            func=mybir.ActivationFunctionType.Identity,
                            bias=b1s[:])
                        ya = flshp.tile([P, HALF], F32, tag="ya")
                        nc.scalar.activation(
                            out=ya[:], in_=y[:],
                            func=mybir.ActivationFunctionType.Identity,
                            scale=a2c[:])
                        nc.vector.tensor_tensor(
                            out=h_fm[:, c0 : c0 + HALF], in0=y[:], in1=ya[:],
                            op=mybir.AluOpType.max)
                    else:
                        yb = flshp.tile([P, HALF], BF16, tag="yb")
                        nc.scalar.activation(
                            out=yb[:], in_=y0[:],
                            func=mybir.ActivationFunctionType.Identity,
                            bias=b2s[:])
                        ot = flshp.tile([P, 2, P], F32, tag="ot")
                        for kk in (0, 1):
                            pt = tp_tp.tile([P, P], BF16, tag="tp")
                            nc.tensor.transpose(
                                out=pt[:], in_=yb[:, kk * P : (kk + 1) * P],
                                identity=idbf[:])
                            nc.any.tensor_copy(out=ot[:, kk, :], in_=pt[:])
                        r0 = c0
                        rows = max(0, min(S - r0, 2 * P))
                        if rows:
                            full, rem = divmod(rows, P)
                            if full:
                                dstp = out[r0 : r0 + full * P, :].rearrange(
                                    "(t p) f -> p t f", p=P)
                                nc.scalar.dma_start(out=dstp, in_=ot[:, :full, :])
                            if rem:
                                nc.scalar.dma_start(
                                    out=out[r0 + full * P : r0 + rows, :],
                                    in_=ot[:rem, full, :])
            return blk

        # ================= schedule both convs =============================
        for conv in (0, 1):
            build_g(conv)
            for q in range(4):
                sender_gathers(conv, q)
                nc.gpsimd.collective_compute(
                    "AllToAll", mybir.AluOpType.bypass,
                    replica_groups=[list(range(NC))],
                    ins=[sendb[conv][q][:]], outs=[recvb[conv][q][:]],
                )
            blk = 0
            for q in range(4):
                blk = scatter_quarter(conv, q, blk)

    from concourse.tile_sem_assignment import PROC_NAME_TO_IDX

    lane_of = {PROC_NAME_TO_IDX[f"DMASW{k}"]: k for k in range(8)}
    for inst in nc.inst_map.values():
        if isinstance(inst, (mybir.InstDMAGatherAnt, mybir.InstDMAScatterAddAnt)):
            proc = getattr(inst, "bass_scheduled_proc", None)
            if proc in lane_of:
                inst.queue_num = lane_of[proc] // 2

    nc.compile()
    return nc


# --------------------------------------------------------------------------
# Host wrapper
# --------------------------------------------------------------------------
def _prep_inputs(x, edge_index, a1, gamma, beta, W1, b1, a2, W2, b2):
    n = x.shape[0]
    S = n // NC
    ei = np.asarray(edge_index).astype(np.int64)
    plan = Plan(ei[0], ei[1], n)
    SP, NT = plan.SP, plan.SP // P

    iota = np.arange(HALF, dtype=np.float32).reshape(1, HALF)
    # eyes[:, :256]: rhs for node tile t0 (1 at (p, p));
    # eyes[:, 256:]: rhs for node tile t0+1 (1 at (p, 128+p)).
    eyes = np.zeros((P, 2 * HALF), dtype=np.float32)
    eyes[np.arange(P), np.arange(P)] = 1.0
    eyes[np.arange(P), HALF + P + np.arange(P)] = 1.0

    in_maps = []
    for c in range(NC):
        dv = plan.dinv[c * S : (c + 1) * S]
        dfm = np.zeros((1, SP), dtype=np.float32)
        dfm[0, :S] = dv
        dvp = np.zeros(SP, dtype=np.float32)
        dvp[:S] = dv
        dnm = np.ascontiguousarray(dvp.reshape(NT, P).T)
        in_maps.append(dict(
            x_sh=np.ascontiguousarray(x[c * S : (c + 1) * S]).astype(np.float32),
            w1=np.asarray(W1, dtype=np.float32),
            w2=np.asarray(W2, dtype=np.float32),
            b1c=np.asarray(b1, dtype=np.float32).reshape(D, 1),
            b2c=np.asarray(b2, dtype=np.float32).reshape(D, 1),
            gam=np.asarray(gamma, dtype=np.float32).reshape(D, 1),
            bet=np.asarray(beta, dtype=np.float32).reshape(D, 1),
            a1=np.asarray(a1, dtype=np.float32).reshape(1, 1),
            a2=np.asarray(a2, dtype=np.float32).reshape(1, 1),
            dinv_nm=dnm,
            dinv_fm=dfm,
            sgidx=plan.send_idx[c],
            dstrel=plan.dstrel[c],
            iota_r=iota,
            eyes=eyes,
        ))
    return plan, in_maps


_PROG_CACHE = {}


def kernel(x, edge_index, a1, gamma, beta, W1, b1, a2, W2, b2, _trace=False):
    x = np.asarray(x)
    n = x.shape[0]
    plan, in_maps = _prep_inputs(x, edge_index, a1, gamma, beta, W1, b1, a2, W2, b2)
    key = plan.key()
    if key not in _PROG_CACHE:
        _PROG_CACHE[key] = build_program(plan)
    nc = _PROG_CACHE[key]
    res = run_bass_kernel_spmd(nc, in_maps, core_ids=list(range(NC)), trace=_trace)
    outs = [res.results[c]["out"] for c in range(NC)]
    full = np.concatenate(outs, axis=0).astype(np.float32)
    kernel._last_exec_ns = res.exec_time_ns
    return full


kernel._last_exec_ns = None
